# revision 11
# baseline (speedup 1.0000x reference)
"""GCN + top-1 MoE kernel for 8 Trainium2 NeuronCores.

Strategy (matches the sharding hint):
- Nodes sharded across the 8 cores (rows of x). Each core computes gating
  (fp32 PE matmul) + all-expert GEMM (bf16) + top-1 combine for its shard,
  producing u = D^-1/2 * h1 in bf16.
- AllGather(u) so every core holds the full u table in local HBM.
- Edges partitioned by destination node: each core aggregates its 12.5k
  destination nodes. Per 128-destination block, incoming edges are gathered
  (dma_gather, 4 source bins to satisfy int16 indices) and summed on the PE
  via one-hot selection matmuls (S^T built on-device with iota+is_equal).
- h2 = relu(D^-1/2 (A+I) u), h3 = h2 @ w2 (PE transpose + bf16 matmul),
  v = D^-1/2 h3, AllGather(v), second aggregation in 64-d space, log_softmax.

Expert weights are replicated (small). All index structures are built on the
host from edge_index only (pure integer preprocessing).
"""

import math
import os

import ml_dtypes
import numpy as np

import concourse.bass as bass
import concourse.mybir as mybir
import concourse.tile as tile
from concourse import bacc
from concourse.bass_utils import run_bass_kernel_spmd
from concourse.masks import make_identity

dt = mybir.dt

M = 8  # cores
NBIN = 4  # source bins (int16 gather index range)

# module-level stash so a test harness can read profiling results
_last_results = None


class _Cfg:
    def __init__(self, n_nodes, f_in, f_h, f_out, n_exp):
        self.N = n_nodes
        self.FIN = f_in
        self.FH = f_h
        self.FOUT = f_out
        self.NEXP = n_exp
        assert n_nodes % M == 0
        self.SH = n_nodes // M
        self.SHP = (self.SH + 127) // 128 * 128
        self.NB = self.SHP // 128
        self.NPG = M * self.SHP
        assert self.NPG % NBIN == 0
        self.BIN = self.NPG // NBIN
        assert self.BIN <= 32767, "int16 gather index range exceeded"
        self.G = min(7, self.NB)  # blocks per gather group
        self.groups = []
        b0 = 0
        while b0 < self.NB:
            g = min(self.G, self.NB - b0)
            self.groups.append((b0, g))
            b0 += g
        assert f_in % 128 == 0
        self.KA = f_in // 128  # contraction chunks for the input GEMMs
        assert f_h % 128 == 0
        self.KH = f_h // 128


def _prep_host(cfg, x, edge_index, w_gate, expert_w, expert_b, bias1, w2, bias2):
    """Pure-numpy preprocessing: normalization scalars, edge binning, packing."""
    N, SH, SHP, NB = cfg.N, cfg.SH, cfg.SHP, cfg.NB

    row = np.asarray(edge_index[0], dtype=np.int64)
    col = np.asarray(edge_index[1], dtype=np.int64)
    deg = (np.bincount(col, minlength=N) + 1).astype(np.float32)
    dinv = deg ** np.float32(-0.5)

    # global padded source index
    gp_all = (row // SH) * SHP + (row % SH)

    # per-core cell assignment: destination shard -> (block, bin)
    K1 = 1
    percore = []
    for c in range(M):
        sel = (col >= c * SH) & (col < (c + 1) * SH)
        s_gp = gp_all[sel]
        d_l = (col[sel] - c * SH).astype(np.int64)
        b_ = d_l >> 7
        p_ = d_l & 127
        q_ = s_gp // cfg.BIN
        l_ = (s_gp % cfg.BIN).astype(np.int64)
        cell = b_ * NBIN + q_
        counts = np.bincount(cell, minlength=NB * NBIN)
        K1 = max(K1, int(math.ceil(counts.max() / 128)))
        percore.append((cell, l_, p_, counts))

    SLOTS = K1 * 128
    NCH = NB * NBIN * K1  # chunk count per core
    core_inputs = []
    for c in range(M):
        cell, l_, p_, counts = percore[c]
        order = np.argsort(cell, kind="stable")
        cell_s = cell[order]
        l_s = l_[order]
        p_s = p_[order]
        off = np.concatenate([[0], np.cumsum(counts)])[:-1]
        j = np.arange(len(cell_s)) - off[cell_s]
        flat = cell_s * SLOTS + j
        gidx_flat = np.zeros(NB * NBIN * SLOTS, dtype=np.int16)
        gidx_flat[flat] = l_s.astype(np.int16)
        dstl_flat = np.full(NB * NBIN * SLOTS, 128, dtype=np.float32)
        dstl_flat[flat] = p_s.astype(np.float32)

        # gidx: one gather per (block, bin) cell; pads are -1 at the cell tail
        gl = gidx_flat.reshape(NB * NBIN, SLOTS)
        gidx = np.concatenate([gl[i].reshape(-1, 16).T for i in range(NB * NBIN)], axis=1)
        gidx = np.tile(gidx, (8, 1))  # [128, NB*NBIN*K1*8]
        gcnt = counts.astype(np.int32)[None, :]  # [1, NB*NBIN] exact valid counts

        # dstl: [128, NCH] with column b*NBIN*K1 + q*K1 + k
        dstl = (
            dstl_flat.reshape(NB, NBIN, K1, 128).transpose(3, 0, 1, 2).reshape(128, NCH)
        )

        # x shard, transposed + padded
        xs = np.zeros((cfg.FIN, SHP), dtype=np.float32)
        xs[:, :SH] = np.asarray(x[c * SH : (c + 1) * SH], dtype=np.float32).T

        dv = np.zeros(SHP, dtype=np.float32)
        dv[:SH] = dinv[c * SH : (c + 1) * SH]
        dinvT = dv.reshape(NB, 128).T.copy()  # [128, NB]

        core_inputs.append(
            {
                "xT": np.ascontiguousarray(xs),
                "gidx": np.ascontiguousarray(gidx),
                "dstl": np.ascontiguousarray(dstl),
                "dinvT": np.ascontiguousarray(dinvT),
            }
        )

    shared = {
        "wg": np.asarray(w_gate, dtype=np.float32),
        "wmoe": np.ascontiguousarray(
            np.asarray(expert_w, dtype=np.float32)
            .transpose(1, 0, 2)
            .reshape(cfg.FIN, cfg.NEXP * cfg.FH)
        ).astype(ml_dtypes.bfloat16),
        "w2": np.asarray(w2, dtype=np.float32).astype(ml_dtypes.bfloat16),
    }
    consts = {
        "eb": np.asarray(expert_b, dtype=np.float32),
        "bias1": np.asarray(bias1, dtype=np.float32),
        "bias2": np.asarray(bias2, dtype=np.float32),
    }
    for m in core_inputs:
        m.update(shared)
    return core_inputs, consts, K1


def _build_program(cfg, K1, consts):
    FIN, FH, FOUT, NEXP = cfg.FIN, cfg.FH, cfg.FOUT, cfg.NEXP
    NB, SHP, NPG, BIN = cfg.NB, cfg.SHP, cfg.NPG, cfg.BIN
    KA, KH = cfg.KA, cfg.KH
    NCH = NB * NBIN * K1
    GCOLS = NCH * 8

    has_eb = bool(np.any(consts["eb"] != 0))
    has_b1 = bool(np.any(consts["bias1"] != 0))
    has_b2 = bool(np.any(consts["bias2"] != 0))
    nhalf = (NEXP * FH + 511) // 512  # 512-wide expert psum slabs

    nc = bacc.Bacc()
    xT_d = nc.dram_tensor("xT", [FIN, SHP], dt.float32, kind="ExternalInput")
    gidx_d = nc.dram_tensor("gidx", [128, GCOLS], dt.int16, kind="ExternalInput")
    dstl_d = nc.dram_tensor("dstl", [128, NCH], dt.float32, kind="ExternalInput")
    dinvT_d = nc.dram_tensor("dinvT", [128, NB], dt.float32, kind="ExternalInput")
    wg_d = nc.dram_tensor("wg", [FIN, NEXP], dt.float32, kind="ExternalInput")
    wmoe_d = nc.dram_tensor("wmoe", [FIN, NEXP * FH], dt.bfloat16, kind="ExternalInput")
    w2_d = nc.dram_tensor("w2", [FH, FOUT], dt.bfloat16, kind="ExternalInput")
    out_d = nc.dram_tensor("out", [SHP, FOUT], dt.float32, kind="ExternalOutput")

    with tile.TileContext(nc) as tc:
        with (
            tc.tile_pool(name="const", bufs=1) as cpool,
            tc.tile_pool(name="ubig", bufs=1) as upool,
            tc.tile_pool(name="x", bufs=3) as xpool,
            tc.tile_pool(name="mg", bufs=12) as mpool,
            tc.tile_pool(name="work", bufs=3) as wpool,
            tc.tile_pool(name="small", bufs=4) as spool,
            tc.tile_pool(name="psb", bufs=2, space="PSUM") as psbpool,
            tc.tile_pool(name="pss", bufs=4, space="PSUM") as psspool,
            tc.tile_pool(name="dram", bufs=1, space="DRAM") as dpool,
        ):
            # ---- resident constants -----------------------------------
            wmoe_sb = cpool.tile([128, KA, NEXP * FH], dt.bfloat16)
            nc.sync.dma_start(wmoe_sb[:], wmoe_d.rearrange("(a p) f -> p a f", p=128))
            wg_sb = cpool.tile([128, KA, NEXP], dt.float32)
            nc.sync.dma_start(wg_sb[:], wg_d.rearrange("(a p) e -> p a e", p=128))
            w2_sb = cpool.tile([128, KH, FOUT], dt.bfloat16)
            nc.sync.dma_start(w2_sb[:], w2_d.rearrange("(h p) o -> p h o", p=128))
            dinvT_sb = cpool.tile([128, NB], dt.float32)
            nc.sync.dma_start(dinvT_sb[:], dinvT_d[:])
            dstl_sb = cpool.tile([128, NCH], dt.float32)
            nc.sync.dma_start(dstl_sb[:], dstl_d[:])
            gidx_sb = cpool.tile([128, GCOLS], dt.int16)
            nc.sync.dma_start(gidx_sb[:], gidx_d[:])
            gcnt_sb = cpool.tile([1, NB * NBIN], dt.int32)
            nc.sync.dma_start(gcnt_sb[:], gcnt_d[:])

            iota_sb = cpool.tile([128, 128], dt.float32)
            nc.gpsimd.iota(iota_sb[:], pattern=[[1, 128]], base=0, channel_multiplier=0, allow_small_or_imprecise_dtypes=True)
            ident_bf = cpool.tile([128, 128], dt.bfloat16)
            make_identity(nc, ident_bf[:])
            if has_eb or has_b1 or has_b2:
                ones_bf = cpool.tile([1, 128], dt.bfloat16)
                nc.vector.memset(ones_bf[:], 1.0)
                bias_sb = cpool.tile([1, NEXP * FH + FH + FOUT], dt.bfloat16)
                # laid out [eb (NEXP*FH) | bias1 (FH) | bias2 (FOUT)] via dram const
                bias_d = nc.dram_tensor(
                    "biases", [1, NEXP * FH + FH + FOUT], dt.bfloat16,
                    kind="ExternalInput",
                )
                nc.sync.dma_start(bias_sb[:], bias_d[:])

            u_sb = upool.tile([128, NB, FH], dt.bfloat16)
            v_sb = upool.tile([128, NB, FOUT], dt.float32)

            u_dram = dpool.tile([SHP, FH], dt.bfloat16)
            u_all = dpool.tile([NPG, FH], dt.bfloat16)
            v_dram = dpool.tile([SHP, FOUT], dt.float32)
            v_all = dpool.tile([NPG, FOUT], dt.float32)

            xT_r = xT_d.rearrange("(a p) n -> p a n", p=128)

            # ---- phase A: gating + experts + combine -> u -------------
            for b in range(NB):
                xb = xpool.tile([128, KA, 128], dt.float32, tag="xb")
                nc.sync.dma_start(xb[:], xT_r[:, :, b * 128 : (b + 1) * 128])
                xbh = xpool.tile([128, KA, 128], dt.bfloat16, tag="xbh")
                nc.vector.tensor_copy(xbh[:], xb[:])

                ps_g = psspool.tile([128, NEXP], dt.float32, tag="pss")
                for a in range(KA):
                    nc.tensor.matmul(
                        ps_g[:], xb[:, a, :], wg_sb[:, a, :],
                        start=(a == 0), stop=(a == KA - 1),
                    )
                ps_y = psbpool.tile([128, nhalf, 512], dt.float32, tag="psb")
                for h in range(nhalf):
                    w0 = h * 512
                    w1 = min((h + 1) * 512, NEXP * FH)
                    for a in range(KA):
                        nc.tensor.matmul(
                            ps_y[:, h, : w1 - w0],
                            xbh[:, a, :],
                            wmoe_sb[:, a, w0:w1],
                            start=(a == 0),
                            stop=(a == KA - 1) and not has_eb,
                        )
                    if has_eb:
                        nc.tensor.matmul(
                            ps_y[:, h, : w1 - w0],
                            ones_bf[:],
                            bias_sb[:, w0:w1],
                            start=False, stop=True,
                        )

                mx = spool.tile([128, 1], dt.float32, tag="mx")
                nc.vector.tensor_reduce(
                    mx[:], ps_g[:], axis=mybir.AxisListType.X, op=mybir.AluOpType.max
                )
                eqm = spool.tile([128, NEXP], dt.float32, tag="eqm")
                nc.vector.tensor_scalar(
                    eqm[:], ps_g[:], mx[:, 0:1], None, op0=mybir.AluOpType.is_equal
                )

                acc0 = wpool.tile([128, FH], dt.float32, tag="acc0")
                acc1 = wpool.tile([128, FH], dt.float32, tag="acc1")

                def _pslice(e):
                    lo = e * FH
                    return ps_y[:, lo // 512, lo % 512 : lo % 512 + FH]

                nc.vector.tensor_scalar(
                    acc0[:], _pslice(0), eqm[:, 0:1], None, op0=mybir.AluOpType.mult
                )
                accs = [acc0, acc1]
                for e in range(1, NEXP):
                    nc.vector.scalar_tensor_tensor(
                        accs[e % 2][:],
                        _pslice(e),
                        eqm[:, e : e + 1],
                        accs[(e - 1) % 2][:],
                        op0=mybir.AluOpType.mult,
                        op1=mybir.AluOpType.add,
                    )
                nc.scalar.activation(
                    u_sb[:, b, :],
                    accs[(NEXP - 1) % 2][:],
                    mybir.ActivationFunctionType.Copy,
                    scale=dinvT_sb[:, b : b + 1],
                )

            nc.sync.dma_start(u_dram[:].rearrange("(nb p) f -> p nb f", p=128), u_sb[:])
            nc.gpsimd.collective_compute(
                "AllGather",
                mybir.AluOpType.bypass,
                replica_groups=[list(range(M))],
                ins=[u_dram.opt()],
                outs=[u_all.opt()],
            )

            # ---- conv1 aggregation + h3 + v ---------------------------
            nidx = K1 * 128
            for b in range(NB):
                    mts = []
                    for q in range(NBIN):
                        cell = b * NBIN + q
                        mt = mpool.tile([128, K1, FH], dt.bfloat16, tag="mt")
                        nc.gpsimd.dma_gather(
                            mt[:],
                            u_all[q * BIN : (q + 1) * BIN, :],
                            gidx_sb[:, cell * nidx // 16 : (cell + 1) * nidx // 16],
                            nidx,
                            nidx,
                            FH,
                        )
                        mts.append(mt)
                    ps_a = psbpool.tile([128, FH], dt.float32, tag="psb")
                    nmm = NBIN * K1 + (1 if has_b1 else 0)
                    i = 0
                    for q in range(NBIN):
                        for k in range(K1):
                            st = spool.tile([128, 128], dt.bfloat16, tag="st")
                            cc = b * (NBIN * K1) + q * K1 + k
                            nc.vector.tensor_scalar(
                                st[:], iota_sb[:], dstl_sb[:, cc : cc + 1], None,
                                op0=mybir.AluOpType.is_equal,
                            )
                            nc.tensor.matmul(
                                ps_a[:], st[:], mts[q][:, bi * K1 + k, :],
                                start=(i == 0), stop=(i == nmm - 1),
                            )
                            i += 1
                    if has_b1:
                        nc.tensor.matmul(
                            ps_a[:], ones_bf[:],
                            bias_sb[:, NEXP * FH : NEXP * FH + FH],
                            start=False, stop=True,
                        )
                    t = wpool.tile([128, FH], dt.float32, tag="evac")
                    nc.vector.tensor_tensor(
                        t[:], ps_a[:], u_sb[:, b, :], op=mybir.AluOpType.add
                    )
                    h2 = wpool.tile([128, FH], dt.bfloat16, tag="h2")
                    nc.scalar.activation(
                        h2[:], t[:], mybir.ActivationFunctionType.Relu,
                        scale=dinvT_sb[:, b : b + 1],
                    )
                    pt = psspool.tile([128, KH, 128], dt.bfloat16, tag="pss")
                    for h in range(KH):
                        nc.tensor.transpose(
                            pt[:, h, :], h2[:, h * 128 : (h + 1) * 128], ident_bf[:]
                        )
                    h2T = wpool.tile([128, KH, 128], dt.bfloat16, tag="h2T")
                    nc.vector.tensor_copy(h2T[:], pt[:])
                    ps_h3 = psspool.tile([128, FOUT], dt.float32, tag="pss")
                    for h in range(KH):
                        nc.tensor.matmul(
                            ps_h3[:], h2T[:, h, :], w2_sb[:, h, :],
                            start=(h == 0), stop=(h == KH - 1),
                        )
                    nc.scalar.activation(
                        v_sb[:, b, :], ps_h3[:],
                        mybir.ActivationFunctionType.Copy,
                        scale=dinvT_sb[:, b : b + 1],
                    )

            nc.sync.dma_start(
                v_dram[:].rearrange("(nb p) f -> p nb f", p=128), v_sb[:]
            )
            nc.gpsimd.collective_compute(
                "AllGather",
                mybir.AluOpType.bypass,
                replica_groups=[list(range(M))],
                ins=[v_dram.opt()],
                outs=[v_all.opt()],
            )

            # ---- conv2 aggregation + log_softmax ----------------------
            for b in range(NB):
                    mts = []
                    for q in range(NBIN):
                        cell = b * NBIN + q
                        mt2 = mpool.tile([128, K1, FOUT], dt.float32, tag="mt")
                        nc.gpsimd.dma_gather(
                            mt2[:],
                            v_all[q * BIN : (q + 1) * BIN, :],
                            gidx_sb[:, cell * nidx // 16 : (cell + 1) * nidx // 16],
                            nidx,
                            nidx,
                            FOUT,
                        )
                        mtb = mpool.tile([128, K1, FOUT], dt.bfloat16, tag="mt")
                        nc.scalar.copy(mtb[:], mt2[:])
                        mts.append(mtb)
                    ps_a = psspool.tile([128, FOUT], dt.float32, tag="pss")
                    nmm = NBIN * K1 + (1 if has_b2 else 0)
                    i = 0
                    for q in range(NBIN):
                        for k in range(K1):
                            st = spool.tile([128, 128], dt.bfloat16, tag="st2")
                            cc = b * (NBIN * K1) + q * K1 + k
                            nc.vector.tensor_scalar(
                                st[:], iota_sb[:], dstl_sb[:, cc : cc + 1], None,
                                op0=mybir.AluOpType.is_equal,
                            )
                            nc.tensor.matmul(
                                ps_a[:], st[:], mts[q][:, bi * K1 + k, :],
                                start=(i == 0), stop=(i == nmm - 1),
                            )
                            i += 1
                    if has_b2:
                        nc.tensor.matmul(
                            ps_a[:], ones_bf[:],
                            bias_sb[:, NEXP * FH + FH :],
                            start=False, stop=True,
                        )
                    h4 = wpool.tile([128, FOUT], dt.float32, tag="h4")
                    nc.vector.tensor_tensor(
                        h4[:], ps_a[:], v_sb[:, b, :], op=mybir.AluOpType.add
                    )
                    h4s = wpool.tile([128, FOUT], dt.float32, tag="h4s")
                    nc.scalar.activation(
                        h4s[:], h4[:], mybir.ActivationFunctionType.Copy,
                        scale=dinvT_sb[:, b : b + 1],
                    )
                    mneg = spool.tile([128, 1], dt.float32, tag="mneg")
                    nc.vector.tensor_reduce(
                        mneg[:], h4s[:], axis=mybir.AxisListType.X,
                        op=mybir.AluOpType.max, negate=True,
                    )
                    es = wpool.tile([128, FOUT], dt.float32, tag="es")
                    ssum = spool.tile([128, 1], dt.float32, tag="ssum")
                    nc.scalar.activation(
                        es[:], h4s[:], mybir.ActivationFunctionType.Exp,
                        bias=mneg[:, 0:1], accum_out=ssum[:],
                    )
                    lse = spool.tile([128, 1], dt.float32, tag="lse")
                    nc.scalar.activation(
                        lse[:], ssum[:], mybir.ActivationFunctionType.Ln
                    )
                    o = wpool.tile([128, FOUT], dt.float32, tag="o")
                    nc.vector.tensor_scalar(
                        o[:], h4s[:], mneg[:, 0:1], lse[:, 0:1],
                        op0=mybir.AluOpType.add, op1=mybir.AluOpType.subtract,
                    )
                    nc.sync.dma_start(out_d[b * 128 : (b + 1) * 128, :], o[:])

    nc.compile()
    return nc


def kernel(x, edge_index, w_gate, expert_w, expert_b, bias1, w2, bias2):
    global _last_results
    x = np.asarray(x)
    edge_index = np.asarray(edge_index)
    cfg = _Cfg(x.shape[0], x.shape[1], np.asarray(expert_w).shape[2],
               np.asarray(w2).shape[1], np.asarray(expert_w).shape[0])

    core_inputs, consts, K1 = _prep_host(
        cfg, x, edge_index, w_gate, expert_w, expert_b, bias1, w2, bias2
    )
    nc = _build_program(cfg, K1, consts)

    if np.any(consts["eb"] != 0) or np.any(consts["bias1"] != 0) or np.any(
        consts["bias2"] != 0
    ):
        biases = np.concatenate(
            [
                consts["eb"].reshape(-1),
                consts["bias1"].reshape(-1),
                consts["bias2"].reshape(-1),
            ]
        ).astype(ml_dtypes.bfloat16)[None, :]
        for mmap in core_inputs:
            mmap["biases"] = biases

    trace = bool(int(os.environ.get("KERNEL_TRACE", "0")))
    res = None
    last_exc = None
    for attempt in range(3):
        try:
            res = run_bass_kernel_spmd(
                nc, core_inputs, core_ids=list(range(M)), trace=trace
            )
            break
        except Exception as e:  # axon worker hiccups are retryable
            last_exc = e
            import time as _time

            _time.sleep(5.0 * (attempt + 1))
    if res is None:
        raise last_exc
    _last_results = res

    out = np.concatenate(
        [res.results[c]["out"][: cfg.SH] for c in range(M)], axis=0
    ).astype(np.float32)
    return out
